# revision 4
# baseline (speedup 1.0000x reference)
"""FFM layer (nn_FFM_Layer) Trainium2 Bass kernel.

Reference computation (B=4096, 13 dense fields, 26 sparse fields with vocab
1000 each, FIELD_NUM=39, K=16):

    idx        = sparse + offsets                      # [B, 26] global ids
    first      = w0 + dense @ w[:13] + sum_j w[idx]    # [B, 1]
    field_f    = einsum('bd,dfk', dense, v[:13]) + sum_j v[idx]   # [B,39,16]
    s          = field_f.sum(1)                        # [B, 16]
    second     = 0.5*(||s||^2 - sum_fk field_f^2)      # [B]
    out        = first + second[:, None]

Strategy (data-parallel over batch, 8 cores x 512 samples, no collectives):
  * Host packs an augmented table V_AUG [26013, 640] f32:
      cols [0:624]  = v.reshape(26013, 39*16)
      col  624      = w[:, 0]   (+ w0 folded into rows of sparse table 0,
                                 which every sample hits exactly once)
      cols [625:640]= 0         (pad so each row is 2560 B, %256 == 0)
  * Each core runs dma_gather (SWDGE, mlp ucode lib) over its 512*26 rows:
    one gathered row brings both the v-row and its w contribution, so a
    single accumulation chain produces field_f AND the sparse w-sum.
  * Dense contribution comes from a [13,128]x[13,640] PE matmul per
    128-sample chunk (dense^T is prepared host-side), which also adds
    dense @ w[:13] via col 624.
  * FM identity epilogue on DVE/ACT per 128-sample chunk.

Memory roofline: 4096*26 rows x 2560 B = 266 MB of gathers, 33.3 MB/core,
~95 us at the ~360 GB/s per-core DMA-bus rate.
"""

import sys

if "/opt/trn_rl_repo" not in sys.path:
    sys.path.insert(0, "/opt/trn_rl_repo")

import numpy as np

import concourse.bacc as bacc
import concourse.bass as bass
import concourse.tile as tile
from concourse import mybir
from concourse.bass_utils import run_bass_kernel_spmd

# Problem constants (hardcoded per harness contract)
B = 4096
N_DENSE = 13
N_SPARSE = 26
FEAT_PER_SPARSE = 1000
FIELD_NUM = 39
FEATURE_NUM = 26013
K = 16
N_CORES = 8
BC = B // N_CORES          # 512 samples per core
ROW = 640                  # padded row: 624 v + 1 w + 15 zeros (2560 B)
VCOLS = FIELD_NUM * K      # 624
P = 128
SCHUNKS = BC // P          # 4 sample chunks of 128 per core
GIDX = BC * N_SPARSE       # 13312 gathered rows per core
# SWDGE descriptor ring caps one gather at ~1024 descriptors on HW
# (1536+ wedges the exec unit), so gather 2 fields (1024 rows) per call.
CHUNK_IDXS = [1024] * 13   # 13 gather calls, 2 fields each
IDX_COLS = sum(n // 16 for n in CHUNK_IDXS)  # 832

F32 = mybir.dt.float32
I16 = mybir.dt.int16


def build_program():
    """Build + compile the single-core SPMD bass program."""
    nc = bacc.Bacc("TRN2", target_bir_lowering=False, debug=False)

    vaug_t = nc.dram_tensor("vaug", [FEATURE_NUM, ROW], F32, kind="ExternalInput")
    dense_t = nc.dram_tensor("dense_t", [N_DENSE, BC], F32, kind="ExternalInput")
    idxs_t = nc.dram_tensor("idxs", [P, IDX_COLS], I16, kind="ExternalInput")
    out_t = nc.dram_tensor("out", [P, SCHUNKS], F32, kind="ExternalOutput")

    with tile.TileContext(nc) as tc:
        with (
            tc.tile_pool(name="main", bufs=1) as main,
            tc.tile_pool(name="gath", bufs=2) as gath,
            tc.tile_pool(name="fold", bufs=2) as fold,
            tc.tile_pool(name="small", bufs=2) as small,
            tc.tile_pool(name="psum", bufs=2, space="PSUM") as psum,
        ):
            idx_sb = main.tile([P, IDX_COLS], I16)
            nc.sync.dma_start(idx_sb[:], idxs_t[:])
            vaug13 = main.tile([N_DENSE, ROW], F32)
            nc.sync.dma_start(vaug13[:], vaug_t[0:N_DENSE, :])
            dt_sb = main.tile([N_DENSE, BC], F32)
            nc.sync.dma_start(dt_sb[:], dense_t[:])

            acc = main.tile([P, SCHUNKS, ROW], F32)
            nc.vector.memset(acc[:], 0.0)
            res = main.tile([P, SCHUNKS], F32)

            # --- gather + accumulate the 26 sparse embeddings ---
            icol = 0
            for ci, n_idx in enumerate(CHUNK_IDXS):
                cols = n_idx // P          # 8 col-chunks of ROW
                g = gath.tile([P, 8, ROW], F32, tag="g")
                nc.gpsimd.dma_gather(
                    g[:, :cols, :],
                    vaug_t[:],
                    idx_sb[:, icol : icol + n_idx // 16],
                    n_idx,
                    n_idx,
                    ROW,
                )
                icol += n_idx // 16
                f2 = fold.tile([P, SCHUNKS, ROW], F32, tag="f2")
                nc.vector.tensor_add(f2[:], g[:, 0:4, :], g[:, 4:8, :])
                nc.vector.tensor_add(acc[:], acc[:], f2[:])

            # --- dense contribution via PE ---
            for c in range(SCHUNKS):
                ps = psum.tile([P, ROW], F32, tag="ps")
                nc.tensor.matmul(
                    out=ps[:, 0:512],
                    lhsT=dt_sb[:, c * P : (c + 1) * P],
                    rhs=vaug13[:, 0:512],
                    start=True,
                    stop=True,
                )
                nc.tensor.matmul(
                    out=ps[:, 512:ROW],
                    lhsT=dt_sb[:, c * P : (c + 1) * P],
                    rhs=vaug13[:, 512:ROW],
                    start=True,
                    stop=True,
                )
                nc.vector.tensor_add(acc[:, c, :], acc[:, c, :], ps[:])

            # --- FM identity epilogue per 128-sample chunk ---
            # (InstTensorTensorReduce wedges the exec unit on this HW path;
            # use ACT Square+accum_out instead.)
            for c in range(SCHUNKS):
                blk = acc[:, c, 0:VCOLS]          # [128, 624] = field_f
                sq = fold.tile([P, VCOLS], F32, tag="sq")
                q = small.tile([P, 1], F32, tag="q")
                nc.scalar.activation(
                    sq[:], blk, mybir.ActivationFunctionType.Square,
                    accum_out=q[:],
                )
                # s-tree: sum 39 fields of 16 -> st[:, 0:16]
                st = fold.tile([P, 320], F32, tag="st")
                # 39 = 19 pairs + 1 leftover -> 20 fields in st
                nc.vector.tensor_add(st[:, 0:304], blk[:, 0:304], blk[:, 304:608])
                nc.vector.tensor_copy(st[:, 304:320], blk[:, 608:624])
                nc.vector.tensor_add(st[:, 0:160], st[:, 0:160], st[:, 160:320])
                nc.vector.tensor_add(st[:, 0:80], st[:, 0:80], st[:, 80:160])
                # 5 fields: 2 pairs + leftover
                nc.vector.tensor_add(st[:, 0:32], st[:, 0:32], st[:, 32:64])
                nc.vector.tensor_add(st[:, 0:16], st[:, 0:16], st[:, 16:32])
                nc.vector.tensor_add(st[:, 0:16], st[:, 0:16], st[:, 64:80])
                s2 = small.tile([P, 16], F32, tag="s2")
                snorm = small.tile([P, 1], F32, tag="snorm")
                nc.scalar.activation(
                    s2[:], st[:, 0:16], mybir.ActivationFunctionType.Square,
                    accum_out=snorm[:],
                )
                # diff = ||s||^2 - ||field_f||^2
                diff = small.tile([P, 1], F32, tag="diff")
                nc.vector.tensor_tensor(
                    out=diff[:], in0=snorm[:], in1=q[:],
                    op=mybir.AluOpType.subtract,
                )
                # out = 0.5*diff + (w-sum incl. w0 and dense first-order)
                nc.scalar.activation(
                    res[:, c : c + 1],
                    diff[:],
                    mybir.ActivationFunctionType.Identity,
                    bias=acc[:, c, VCOLS : VCOLS + 1],
                    scale=0.5,
                )

            nc.sync.dma_start(out_t[:], res[:])

    nc.compile()
    return nc


def prep_inputs(dense_inputs, sparse_inputs, w0, w, v):
    """Host-side shard/pack: build per-core in_maps."""
    dense = np.asarray(dense_inputs, np.float32)
    sparse = np.asarray(sparse_inputs)
    w0 = np.asarray(w0, np.float32)
    w = np.asarray(w, np.float32)
    v = np.asarray(v, np.float32)

    vaug = np.zeros((FEATURE_NUM, ROW), np.float32)
    vaug[:, :VCOLS] = v.reshape(FEATURE_NUM, VCOLS)
    vaug[:, VCOLS] = w[:, 0]
    # fold w0 into sparse table 0 (each sample hits it exactly once)
    vaug[N_DENSE : N_DENSE + FEAT_PER_SPARSE, VCOLS] += w0[0]

    offs = N_DENSE + FEAT_PER_SPARSE * np.arange(N_SPARSE, dtype=np.int64)
    gidx = (sparse.astype(np.int64) + offs[None, :]).astype(np.int16)  # [B, 26]

    in_maps = []
    for c in range(N_CORES):
        sl = slice(c * BC, (c + 1) * BC)
        dt = np.ascontiguousarray(dense[sl].T)          # [13, 512]
        flat = np.ascontiguousarray(gidx[sl].T).reshape(-1)  # j-major [13312]
        buf = np.zeros((P, IDX_COLS), np.int16)
        off_i = 0
        off_c = 0
        for n in CHUNK_IDXS:
            seg = flat[off_i : off_i + n]
            wrapped = seg.reshape(n // 16, 16).T        # [16, n/16]
            buf[:, off_c : off_c + n // 16] = np.tile(wrapped, (8, 1))
            off_i += n
            off_c += n // 16
        in_maps.append({"vaug": vaug, "dense_t": dt, "idxs": buf})
    return in_maps


_NC_CACHE = None


def kernel(dense_inputs, sparse_inputs, w0, w, v):
    global _NC_CACHE
    if _NC_CACHE is None:
        _NC_CACHE = build_program()
    nc = _NC_CACHE
    in_maps = prep_inputs(dense_inputs, sparse_inputs, w0, w, v)
    res = run_bass_kernel_spmd(nc, in_maps, core_ids=list(range(N_CORES)))
    outs = []
    for r in res.results:
        o = r["out"]                                    # [128, 4]
        outs.append(np.ascontiguousarray(o.T).reshape(BC, 1))
    return np.concatenate(outs, axis=0).astype(np.float32)
